# revision 44
# baseline (speedup 1.0000x reference)
"""Trainium2 Bass kernel for multi-head attention (B=2, S=2048, D=2048, 16 heads).

Sharding: 8 cores = 2 batch groups (data parallel) x 4 tensor-parallel ranks.
Each core handles one batch element and 4 heads (wqkv rows / wo cols sharded by
head). Partial output projections are summed with a ReduceScatter per
query chunk over each 4-core group; the host reassembles the full output.

Key layout decisions (v1 restructure):
- All matmul inputs pre-transposed host-side: xt = x^T, wqkvt = wqkv_shard^T,
  wot = wo_shard^T. Head-dim of q/k permuted (even,odd) so RoPE pairs are
  partition j / 64+j in the [hd, s] layout.
- Scores are computed TRANSPOSED [k, q] (lhsT = k-tile, rhs = q-chunk) so the
  exp() output is already in the PV orientation -> no PE transposes of P.
- Softmax denominators via a ones-matmul accumulated in PSUM alongside PV;
  normalization = reciprocal + one tensor_tensor multiply per (head, chunk).
- V is computed directly in [s, hd] layout (lhsT = x-tile, rhs = w_v cols).
- Causal diag-block mask generated on device (gpsimd affine_select).
- DMAs: one big transfer per dt-tile (wqkvt 384KB, xt 512KB), interleaved so
  the first QKV matmul starts as soon as dt=0 lands.
- RoPE: 2 full-width vector TTs per chunk against stacked [c;c]/[s;-s]
  tables; the cross-partition combines ride gpsimd DGE accum-DMAs.
- Attention is software-pipelined with a 1-kt skew (scores/exp of kt emitted
  before ones/PV of kt-1) so the PE never waits on the scalar-engine exp.
- Softmax reciprocal uses reciprocal_approx_fast (5x faster, 18-bit).
- One ReduceScatter per 512-row chunk; a tiny dummy RS during phase A absorbs
  the ~11us first-collective trigger warmup; mid-kernel output copies ride
  the gpsimd DGE queue so they never head-of-line block the sync DMA queue.
"""

import sys
import numpy as np
import ml_dtypes

sys.path.insert(0, "/opt/trn_rl_repo")

B, S, D = 2, 2048, 2048
NH, HD = 16, 128
TP = 4            # tensor-parallel ranks per batch group
HL = NH // TP     # heads per core = 4
DL = HL * HD      # local out-proj contraction = 512
NDT = D // 128    # 16 d-tiles
NQT = S // 128    # 16 q/k tiles
NQC = 4           # 512-row query chunks
SM_SCALE = float(HD) ** -0.5
GROUPS = [[0, 1, 2, 3], [4, 5, 6, 7]]

_cache = {}


def _build_graph():
    import concourse.bass as bass
    import concourse.mybir as mybir
    import concourse.tile as tile
    from concourse import bacc

    f32 = mybir.dt.float32
    bf16 = mybir.dt.bfloat16
    AF = mybir.ActivationFunctionType
    OP = mybir.AluOpType

    nc = bacc.Bacc("TRN2", target_bir_lowering=False, debug=False, num_devices=8)

    xt_ext = nc.declare_dram_parameter("xt", [D, S], bf16, isOutput=False)
    wqkvt_ext = nc.declare_dram_parameter("wqkvt", [D, 3 * DL], bf16, isOutput=False)
    wot_ext = nc.declare_dram_parameter("wot", [DL, D], bf16, isOutput=False)
    # stacked rotary tables: cost2 = [c; c], sint2 = [s; -s]  (see RoPE below)
    cost_ext = nc.declare_dram_parameter("cost", [HD, S], bf16, isOutput=False)
    sint_ext = nc.declare_dram_parameter("sint", [HD, S], bf16, isOutput=False)
    out_ext = nc.declare_dram_parameter("out", [512, D], bf16, isOutput=True)

    with tile.TileContext(nc) as tc:
        with tc.tile_pool(name="pers", bufs=1) as pers, \
             tc.tile_pool(name="dram", bufs=1, space="DRAM") as dram:
            # persistent tensors used by attention
            qk_bf = [pers.tile([128, S], bf16, tag=f"qk{i}", name=f"qk{i}")
                     for i in range(2 * HL)]            # 4 q heads then 4 k heads, [hd, s]
            v_bf = [pers.tile([128, DL], bf16, tag=f"v{i}", name=f"v{i}")
                    for i in range(NQT)]                # [s-tile, 4*hd']
            wo_bf = [pers.tile([128, D], bf16, tag=f"wo{h}", name=f"wo{h}")
                     for h in range(HL)]
            ones_bf = pers.tile([128, 128], bf16, tag="ones", name="ones")
            mask_sb = pers.tile([128, 128], f32, tag="mask", name="mask")

            # on-device constants (no DMA)
            nc.vector.memset(ones_bf[:], 1.0)
            nc.gpsimd.memset(mask_sb[:], 0.0)
            # keep 0 where q - k >= 0 (x=k partition, y=q free), else -30000
            nc.gpsimd.affine_select(
                out=mask_sb[:], in_=mask_sb[:],
                compare_op=OP.is_ge, fill=-30000.0,
                base=0, pattern=[[1, 128]], channel_multiplier=-1)

            # ---------------- Phase A: QKV projection + RoPE ----------------
            with tc.tile_pool(name="early", bufs=1) as early:
                xt_bf = [early.tile([128, S], bf16, tag=f"xt{i}", name=f"xt{i}")
                         for i in range(NDT)]
                w_sb = [early.tile([128, 3 * DL], bf16, tag=f"w{i}", name=f"w{i}")
                        for i in range(NDT)]
                ck = early.tile([HD, S], bf16, tag="ck", name="ck")
                sk = early.tile([HD, S], bf16, tag="sk", name="sk")

                # interleaved big DMAs: first matmul only needs dt=0 pair
                for dt in range(NDT):
                    nc.sync.dma_start(out=w_sb[dt][:],
                                      in_=wqkvt_ext[dt * 128:(dt + 1) * 128, :])
                    nc.sync.dma_start(out=xt_bf[dt][:],
                                      in_=xt_ext[dt * 128:(dt + 1) * 128, :])
                    if dt == 3:
                        nc.sync.dma_start(out=ck[:], in_=cost_ext[:])
                        nc.sync.dma_start(out=sk[:], in_=sint_ext[:])
                # wo needed only at out-proj time
                for h in range(HL):
                    nc.sync.dma_start(out=wo_bf[h][:],
                                      in_=wot_ext[h * 128:(h + 1) * 128, :])

                # dummy tiny RS early: absorbs the ~11us first-collective
                # trigger warmup so real RS ops start fast
                dumi = dram.tile([4, 16], bf16, tag="dumi", name="dumi")
                dumo = dram.tile([1, 16], bf16, tag="dumo", name="dumo")
                nc.gpsimd.collective_compute(
                    "ReduceScatter", OP.add, replica_groups=GROUPS,
                    ins=[dumi[:].opt()], outs=[dumo[:].opt()])

                # Q/K in [e, s] layout: lhsT = wqkvt col-slice, rhs = xt tile
                with tc.tile_pool(name="psQK", bufs=1, space="PSUM") as psQK, \
                     tc.tile_pool(name="rope", bufs=1) as rope_pool:
                    # k0,k1,q0,q1 first so head-pair (0,1) attention unblocks
                    # early; the V matmuls below shadow the RoPE tail
                    for ei, et in enumerate((4, 5, 0, 1, 6, 7, 2, 3)):
                        ps_qk = psQK.tile([128, S], f32, tag=f"ps_qk{ei % 2}",
                                          name=f"ps_qk{et}", bufs=1)
                        for dt in range(NDT):
                            for sc in range(4):
                                nc.tensor.matmul(
                                    ps_qk[:, sc * 512:(sc + 1) * 512],
                                    w_sb[dt][:, et * 128:(et + 1) * 128],
                                    xt_bf[dt][:, sc * 512:(sc + 1) * 512],
                                    start=(dt == 0), stop=(dt == NDT - 1))
                        # RoPE via stacked tables, 2 TTs + 2 DMA-accum per
                        # chunk. With r=rows 0:64, i=rows 64:128 of psum:
                        #   m1 = psum*[c;c]  = [r*c ; i*c] -> written to qk
                        #   m2 = psum*[s;-s] = [r*s ; -i*s]
                        #   qk[0:64]  += m2[64:128]  (= r*c - i*s)
                        #   qk[64:128]+= m2[0:64]    (= i*c + r*s)
                        # The cross-partition adds ride the DMA engines
                        # (accum_op), keeping vector work at 2 TTs/chunk.
                        t2 = rope_pool.tile([128, S], bf16, tag="t2",
                                            name=f"t_{et}", bufs=2)
                        nc.vector.tensor_tensor(out=qk_bf[et][:], in0=ps_qk[:],
                                                in1=ck[:], op=OP.mult)
                        nc.vector.tensor_tensor(out=t2[:], in0=ps_qk[:],
                                                in1=sk[:], op=OP.mult)
                        nc.gpsimd.dma_start(out=qk_bf[et][0:64, :],
                                            in_=t2[64:128, :],
                                            accum_op=OP.add)
                        nc.gpsimd.dma_start(out=qk_bf[et][64:128, :],
                                            in_=t2[0:64, :],
                                            accum_op=OP.add)

                # V directly in [s, hd'] layout: lhsT = xt s-slice, rhs = w_v cols
                with tc.tile_pool(name="psV", bufs=1, space="PSUM") as psV:
                    for st in range(NQT):
                        ps_v = psV.tile([128, DL], f32, tag=f"psv{st % 4}",
                                        name=f"psv{st}", bufs=1)
                        for dt in range(NDT):
                            nc.tensor.matmul(
                                ps_v[:],
                                xt_bf[dt][:, st * 128:(st + 1) * 128],
                                w_sb[dt][:, 2 * DL:3 * DL],
                                start=(dt == 0), stop=(dt == NDT - 1))
                        nc.scalar.copy(v_bf[st][:], ps_v[:])

            # ---------------- Phase B: attention + out-proj + RS ----------------
            with tc.tile_pool(name="att", bufs=1) as att, \
                 tc.tile_pool(name="psB", bufs=1, space="PSUM") as psB:
                o2_bf = [att.tile([128, S], bf16, tag=f"o2{h}", name=f"o2{h}")
                         for h in range(HL)]            # attn out per head, [hd, q]

                def attn_pair(qc, pair):
                    # software-pipelined with 1-kt skew: scores/exp for kt are
                    # emitted before ones/PV matmuls of kt-1, so the PE never
                    # waits on the scalar-engine exp.
                    nkt = qc * 4 + 4
                    o2_ps = {}
                    sums_ps = {}
                    ex_prev = {}
                    for h in pair:
                        o2_ps[h] = psB.tile([128, 512], f32, tag=f"o2p{h % 2}",
                                            name=f"o2p_{qc}_{h}", bufs=2)
                        sums_ps[h] = psB.tile([128, 512], f32, tag=f"sm{h % 2}",
                                              name=f"sm_{qc}_{h}", bufs=1)

                    def n_of(kt):
                        return 512 - max(kt - qc * 4, 0) * 128

                    for kt in range(nkt + 1):
                        if kt < nkt:
                            j = kt - qc * 4      # >=0 on the diagonal k-tiles
                            off = 512 - n_of(kt)
                            n = n_of(kt)
                            ps_sc = {}
                            ex = {}
                            for h in pair:
                                ps_sc[h] = psB.tile(
                                    [128, 512], f32, tag=f"sc{h % 2}",
                                    name=f"sc_{qc}_{h}_{kt}", bufs=1)
                                nc.tensor.matmul(
                                    ps_sc[h][:, 0:n],
                                    qk_bf[HL + h][:, kt * 128:(kt + 1) * 128],
                                    qk_bf[h][:, qc * 512 + off:(qc + 1) * 512],
                                    start=True, stop=True)
                            if j >= 0:
                                for h in pair:
                                    nc.vector.tensor_tensor(
                                        out=ps_sc[h][:, 0:128],
                                        in0=ps_sc[h][:, 0:128],
                                        in1=mask_sb[:], op=OP.add)
                            for h in pair:
                                ex[h] = att.tile([128, 512], bf16,
                                                 tag=f"ex{h % 2}",
                                                 name=f"ex_{qc}_{h}_{kt}", bufs=3)
                                nc.scalar.activation(
                                    ex[h][:, 0:n], ps_sc[h][:, 0:n], AF.Exp,
                                    scale=SM_SCALE)
                        if kt > 0:
                            pk = kt - 1
                            off = 512 - n_of(pk)
                            n = n_of(pk)
                            for h in pair:
                                nc.tensor.matmul(
                                    sums_ps[h][:, off:512], ones_bf[:],
                                    ex_prev[h][:, 0:n],
                                    start=(pk == 0), stop=(pk == nkt - 1))
                                nc.tensor.matmul(
                                    o2_ps[h][:, off:512],
                                    v_bf[pk][:, (h % HL) * 128:((h % HL) + 1) * 128],
                                    ex_prev[h][:, 0:n],
                                    start=(pk == 0), stop=(pk == nkt - 1))
                        if kt < nkt:
                            ex_prev = ex
                    for h in pair:
                        rec = att.tile([128, 512], f32, tag=f"rec{h % 2}",
                                       name=f"rec_{qc}_{h}", bufs=1)
                        nc.vector.reciprocal_approx_fast(rec[:], sums_ps[h][:])
                        nc.vector.tensor_tensor(
                            out=o2_bf[h][:, qc * 512:(qc + 1) * 512],
                            in0=o2_ps[h][:], in1=rec[:], op=OP.mult)

                def outproj(qc):
                    # partial out-projection for this chunk + ReduceScatter.
                    # pr tiles ping-pong on the sums-psum tags (free by now).
                    bnc = dram.tile([512, D], bf16, tag=f"bnc{qc}", name=f"bnc{qc}")
                    for st_l in range(4):
                        st = qc * 4 + st_l
                        fin = att.tile([128, D], bf16, tag="fin",
                                       name=f"fin_{qc}_{st_l}", bufs=2)
                        for ec in range(4):
                            ps_pr = psB.tile([128, 512], f32, tag=f"sm{ec % 2}",
                                             name=f"pr_{qc}_{st_l}_{ec}", bufs=1)
                            for h in range(HL):
                                nc.tensor.matmul(
                                    ps_pr[:],
                                    o2_bf[h][:, st * 128:(st + 1) * 128],
                                    wo_bf[h][:, ec * 512:(ec + 1) * 512],
                                    start=(h == 0), stop=(h == HL - 1))
                            nc.vector.tensor_copy(fin[:, ec * 512:(ec + 1) * 512],
                                                  ps_pr[:])
                        nc.sync.dma_start(
                            out=bnc[st_l * 128:(st_l + 1) * 128, :], in_=fin[:])
                    rso = dram.tile([128, D], bf16, tag=f"rso{qc}",
                                    name=f"rso{qc}")
                    nc.gpsimd.collective_compute(
                        "ReduceScatter", OP.add,
                        replica_groups=GROUPS,
                        ins=[bnc[:].opt()],
                        outs=[rso.opt()])
                    if qc < NQC - 1:
                        # defer: a DGE copy emitted here would sit between RS
                        # triggers on the gpsimd queue and delay the next RS
                        pending_out.append((qc, rso))
                    else:
                        # tail copy on the fast hardware DMA queue
                        nc.sync.dma_start(
                            out=out_ext[qc * 128:(qc + 1) * 128, :], in_=rso[:])

                pending_out = []
                for qc in range(NQC):
                    attn_pair(qc, (0, 1))
                    attn_pair(qc, (2, 3))
                    outproj(qc)
                # mid-kernel output copies ride the gpsimd DGE, emitted after
                # every RS trigger so they never delay the collective stream
                for (oqc, orso) in pending_out:
                    nc.gpsimd.dma_start(
                        out=out_ext[oqc * 128:(oqc + 1) * 128, :], in_=orso[:])
    nc.finalize()
    return nc


def _prep_inputs(x, freqs_cos, freqs_sin, mask, wqkv, wo):
    bf = ml_dtypes.bfloat16
    perm = np.concatenate([np.arange(0, HD, 2), np.arange(1, HD, 2)])
    cos_t = np.asarray(freqs_cos, np.float32).T      # [HD//2, S]
    sin_t = np.asarray(freqs_sin, np.float32).T
    cost = np.ascontiguousarray(np.concatenate([cos_t, cos_t], axis=0)).astype(bf)
    sint = np.ascontiguousarray(np.concatenate([sin_t, -sin_t], axis=0)).astype(bf)
    wqkv = np.asarray(wqkv, np.float32)
    wo = np.asarray(wo, np.float32)
    x = np.asarray(x, np.float32)

    in_maps = []
    for c in range(8):
        b, r = divmod(c, TP)
        heads = range(r * HL, (r + 1) * HL)
        rows = []
        for sec in range(2):  # q then k, head-dim permuted
            for h in heads:
                blk = wqkv[sec * D + h * HD: sec * D + (h + 1) * HD]
                rows.append(blk[perm])
        for h in heads:       # v, natural order
            rows.append(wqkv[2 * D + h * HD: 2 * D + (h + 1) * HD])
        wqkv_shard = np.concatenate(rows, axis=0)           # [1536, 2048]
        wqkvt = np.ascontiguousarray(wqkv_shard.T).astype(bf)
        wo_shard = np.concatenate(
            [wo[:, h * HD:(h + 1) * HD] for h in heads], axis=1)  # [2048, 512]
        wot = np.ascontiguousarray(wo_shard.T).astype(bf)
        xt = np.ascontiguousarray(x[b].T).astype(bf)
        in_maps.append({
            "xt": xt, "wqkvt": wqkvt, "wot": wot,
            "cost": cost, "sint": sint,
        })
    return in_maps


def kernel(x, freqs_cos, freqs_sin, mask, wqkv, wo, input_pos=None,
           _want_res=False, _trace=False, _tmpdir=None):
    from concourse.bass_utils import run_bass_kernel_spmd

    if "nc" not in _cache:
        _cache["nc"] = _build_graph()
    nc = _cache["nc"]

    in_maps = _prep_inputs(x, freqs_cos, freqs_sin, mask, wqkv, wo)
    kw = {}
    if _trace:
        kw = dict(trace=True, tmpdir=_tmpdir)
    res = run_bass_kernel_spmd(nc, in_maps, list(range(8)), **kw)

    y = np.empty((B, S, D), np.float32)
    for c in range(8):
        b, r = divmod(c, TP)
        oc = np.asarray(res.results[c]["out"], np.float32)
        for qc in range(NQC):
            y[b, qc * 512 + r * 128: qc * 512 + (r + 1) * 128, :] = \
                oc[qc * 128:(qc + 1) * 128]
    if _want_res:
        return y, res
    return y
